# revision 49
# baseline (speedup 1.0000x reference)
"""DecoderRNN Trainium2 kernel (v4).

Math (reference):
    emb = embed_table[captions]                      # (B, 31, E)
    inputs = concat([features[:,None,:], emb], 1)    # (B, T=32, E)
    xproj = inputs @ Wi + (bi + bh)                  # (B, T, H)
    h_t = tanh(xproj_t + h_{t-1} @ Wh)               # scan over T
    out = hs @ Wy + by                               # (B, T, V)

Distribution: vocab-parallel output projection across 8 cores (Wy sharded by
1250 columns); the input GEMM and serial RNN are replicated full-batch on
every core. No collectives. `by` is added on the HOST during assemble, so
every projection psum evacuation is a plain dtype-converting copy that can
run on either DVE or ACT.

v4 = v2's chain structure + three fixes derived from the v2/v3 traces:
  * v2's tail (36.5us at 66% PE busy) was caused by the projection sharing
    one psum pool with the xblock tails: pool slots recycle in allocation
    order, so every projection chunk transitively waited on the t=25 xblock.
    v4 gives the projection its own 4-bank pool (+2 xblock, +2 RNN), and
    emits it after the chain (higher bass_priority number = filler), which
    lets the greedy scheduler pack it into chain idle from step ~3 on.
    (v3 showed the opposite order head-blocks the chain: interleaving the
    projection BEFORE later chain steps gives it lower priority and the PE
    grinds filler while ACT waits.)
  * Evacuation split: DVE carries most psum->sbuf copies, ACT takes chunks
    of the last tiles (tanhs are emitted earlier == lower priority, so ACT
    always prefers the chain). GPSIMD cannot read PSUM at all.
  * Input DMAs split across both HWDGE queues: Sync carries chain-critical
    tensors (wi, inp_head, ident, bias, wh -- wh now BEFORE the bulk, v2
    lost ~2us of chain stall to wh landing after in_rest), ACT carries the
    bulk (in_rest, tails, wy). Output stores (1 per tile) go on Sync.
  * RNN step: identity matmul streams xpT[t-1] into psum (start=True over
    all four [128,64] m-regions), 16 Wh matmuls accumulate, one fused tanh
    (ACT) evacuates psum to hsT. Serial chain is PE->ACT->PE (~1.27us/step),
    which stays under the PE-throughput-bound steady-state cadence.
  * ~10 garbage warmup matmuls bridge the preamble->first-DMA window so the
    PE clock gate is at full rate when the real GEMMs start.
  * fp16 operands everywhere (peak 0.417 ns/col); psum accumulates fp32;
    fp16 output (host upcasts and adds by).

On-chip layout keeps H (or E) on the partition axis everywhere:
    inputsT  [128, nb, k, c]   k = E/128 chunk, c = bt col within nb block
    xpT      [128, t, m*64+b]  m = H/128 chunk of the output
    hsT      [128, m, t*64+b]  t = 1..32 (slot 0 unused)
"""

import sys

sys.path.insert(0, "/opt/trn_rl_repo")

from contextlib import ExitStack

import numpy as np

import concourse.bass as bass
import concourse.mybir as mybir
import concourse.tile as tile
from concourse import bacc
from concourse.bass import ts
from concourse.bass_utils import run_bass_kernel_spmd

B, T, E, H, V = 64, 32, 512, 512, 10000
NCORES = 8
VS = V // NCORES          # vocab shard per core
BT = B * T                # 2048 rows, t-major: row = t*64 + b
P = 128
KE = E // P               # 4 contraction chunks over E
KH = H // P               # 4 contraction chunks over H
MT = H // P               # 4 output chunks of H
NB = 4                    # bt blocks of 512 for the input GEMM
F32 = mybir.dt.float32
F16 = mybir.dt.float16
HOST_BY = True            # by is added on the host in assemble()

# projection N-chunks (psum bank holds 512 fp32 per partition)
VCHUNKS = [(0, 512), (512, 512), (1024, 226)]
assert sum(n for _, n in VCHUNKS) == VS


def build_program() -> bass.Bass:
    nc = bacc.Bacc()

    inp_head = nc.dram_tensor("inp_head", [P, KE, 128], F16, kind="ExternalInput")
    inp_resta = nc.dram_tensor("inp_resta", [P, KE, 128], F16, kind="ExternalInput")
    inp_restb = nc.dram_tensor("inp_restb", [P, KE, 256], F16, kind="ExternalInput")
    inp_tail = nc.dram_tensor("inp_tail", [P, NB - 1, KE, 512], F16, kind="ExternalInput")
    wi = nc.dram_tensor("wi", [P, KE, H], F16, kind="ExternalInput")
    wh = nc.dram_tensor("wh", [P, KH, MT, P], F16, kind="ExternalInput")
    bias = nc.dram_tensor("bias", [P, MT], F32, kind="ExternalInput")  # bi + bh
    wy = nc.dram_tensor("wy", [P, KH, VS], F16, kind="ExternalInput")
    ident = nc.dram_tensor("ident", [P, P], F16, kind="ExternalInput")
    out = nc.dram_tensor("out", [BT, VS], F16, kind="ExternalOutput")

    with ExitStack() as ctx:
        tc = ctx.enter_context(tile.TileContext(nc))
        persist = ctx.enter_context(tc.tile_pool(name="persist", bufs=1))
        out_pool = ctx.enter_context(tc.tile_pool(name="outs", bufs=6))
        proj_a = ctx.enter_context(tc.tile_pool(name="pj_a", bufs=3, space="PSUM"))
        proj_b = ctx.enter_context(tc.tile_pool(name="pj_b", bufs=3, space="PSUM"))
        rnn_psum = ctx.enter_context(tc.tile_pool(name="rn_ps", bufs=2, space="PSUM"))

        # ---- One sync HWDGE queue, ordered by need-time (the two HWDGE
        # queues share DMA bandwidth, and >8 in-flight DMAs stall the issue
        # pipeline on semaphore reuse, so: 8 issues on sync, and the tiny
        # ident/bias via the gpsimd software-DGE queue).
        wi_sb = persist.tile([P, KE, H], F16, tag="wi")
        in_head = persist.tile([P, KE, 128], F16, tag="in_head")
        ident_sb = persist.tile([P, P], F16, tag="ident")
        bias_sb = persist.tile([P, MT], F32, tag="bias")
        wh_sb = persist.tile([P, KH, MT, P], F16, tag="wh")
        in_resta = persist.tile([P, KE, 128], F16, tag="in_resta")
        in_restb = persist.tile([P, KE, 256], F16, tag="in_restb")
        in_tail = persist.tile([P, NB - 1, KE, 512], F16, tag="in_tail")
        wy_sb = persist.tile([P, KH, VS], F16, tag="wy")
        nc.sync.dma_start(out=wi_sb[:, 0:2], in_=wi[:, 0:2])
        nc.sync.dma_start(out=in_head[:], in_=inp_head[:])
        nc.sync.dma_start(out=wi_sb[:, 2:4], in_=wi[:, 2:4])
        nc.sync.dma_start(out=ident_sb[:], in_=ident[:])
        nc.sync.dma_start(out=bias_sb[:], in_=bias[:])
        nc.sync.dma_start(out=in_resta[:], in_=inp_resta[:])
        nc.sync.dma_start(out=wh_sb[:], in_=wh[:])
        nc.sync.dma_start(out=in_restb[:], in_=inp_restb[:])
        for nb in range(NB - 1):
            nc.sync.dma_start(out=in_tail[:, nb, :, :], in_=inp_tail[:, nb, :, :])
        nc.sync.dma_start(out=wy_sb[:], in_=wy[:])

        # ---- HAM warmup: the PE clock-gate throttles to half rate when the
        # array is idle; keep it busy on garbage during the DMA wait.
        warm = persist.tile([P, 512], F16, tag="warm")
        nc.vector.memset(warm[:], 0.0)
        wps = proj_a.tile([P, 512], F32, tag="gemm")
        for _ in range(6):
            nc.tensor.matmul(
                wps[:], lhsT=warm[:, 0:P], rhs=warm[:], start=True, stop=True,
                skip_group_check=True,
            )

        xpT = persist.tile([P, T, MT * B], F16, tag="xpT")
        hsT = persist.tile([P, MT, (T + 1) * B], F16, tag="hsT")

        # ---- projection emitter: out[bt_tile] = hs @ Wy (by added on
        # host), broken into single matmuls so the chain loop can emit a
        # measured 3 per step (fits the real ~740ns tanh window; the Tile
        # scheduler bakes its simulated order into semaphores, so packing
        # must be done at emission). In-chain chunks all use stream A
        # (psum pool proj_a + DVE evac, keeping ACT free for tanhs); the
        # post-chain drain alternates two independent pool+engine streams
        # so one stream's psum-release latency hides under the other.
        pstate = {"pp": None, "osb": None, "cnt": 0}

        def proj_next_cost():
            i, rem = divmod(pstate["cnt"], 12)
            if i >= BT // P:
                return None
            return VCHUNKS[rem // KH][1] * 0.417 + 2.0

        def emit_proj_mm(in_chain):
            cnt = pstate["cnt"]
            i, rem = divmod(cnt, 12)          # 3 chunks x 4 k per tile
            ci, k = divmod(rem, KH)
            if i >= BT // P:
                return False
            v0, vn = VCHUNKS[ci]
            # in-chain chunks stay on stream A (DVE evac; ACT is doing the
            # chain tanhs); post-chain chunks rotate over THREE pools -- the
            # rnn banks are dead after the chain, so they serve a 3rd stream
            st = 0 if in_chain else (i * 3 + ci) % 3
            if k == 0:
                if ci == 0:
                    pstate["osb"] = out_pool.tile(
                        [P, VS], F16, tag="osb", name="osb"
                    )
                if st == 2:
                    pstate["pp"] = rnn_psum.tile(
                        [P, 512], F32, tag="rnn", name="pp"
                    )
                else:
                    pstate["pp"] = (proj_a if st == 0 else proj_b).tile(
                        [P, 512], F32, tag="gemm", name="pp"
                    )
            pp, osb = pstate["pp"], pstate["osb"]
            nc.tensor.matmul(
                pp[:, :vn],
                lhsT=hsT[:, k, (2 * i + 1) * B : (2 * i + 1) * B + P],
                rhs=wy_sb[:, k, v0 : v0 + vn],
                start=(k == 0),
                stop=(k == KH - 1),
            )
            if k == KH - 1:
                if st == 1:
                    nc.scalar.copy(osb[:, v0 : v0 + vn], pp[:, :vn])
                else:
                    nc.vector.tensor_scalar_add(
                        osb[:, v0 : v0 + vn], pp[:, :vn], 0.0
                    )
                if i < BT // P - 1:
                    if ci == 2:
                        nc.sync.dma_start(out=out[ts(i, P), :], in_=osb[:])
                else:
                    # last tile: store each chunk as it lands so the final
                    # transfer is the short 226-col one
                    nc.sync.dma_start(
                        out=out[ts(i, P), v0 : v0 + vn], in_=osb[:, v0 : v0 + vn]
                    )
            pstate["cnt"] = cnt + 1
            return True

        # ---- xprojT = (inputs @ Wi).T + (bi + bh), in bt-blocks; evacuated
        # by DVE (tensor_scalar add of the per-partition bias chunk). The
        # m-chunks alternate between the two psum pools so the real ~1.7us
        # psum-release turnaround never stalls an xblock.
        xcnt = [0]

        def xblock(rhs_tile, nb, t0, cn):
            for m in range(MT):
                ps = (proj_a if xcnt[0] % 2 == 0 else proj_b).tile(
                    [P, 512], F32, tag="gemm", name="ps"
                )
                xcnt[0] += 1
                for k in range(KE):
                    nc.tensor.matmul(
                        ps[:, :cn],
                        lhsT=wi_sb[:, k, ts(m, P)],
                        rhs=rhs_tile[:, k, :] if nb is None else rhs_tile[:, nb, k, :],
                        start=(k == 0),
                        stop=(k == KE - 1),
                    )
                nc.vector.tensor_scalar_add(
                    xpT[:, t0 : t0 + cn // B, ts(m, B)],
                    ps[:, :cn].rearrange("p (t b) -> p t b", b=B),
                    bias_sb[:, m : m + 1],
                )

        # tail-block xblocks as single-matmul emitters so the chain loop can
        # drip 3 per step (a 16-matmul dump at one step stalls the chain
        # ~2.5-4us; the sim's tighter chain model can't hide it)
        xb_state = {}

        def emit_xb_mm(nb):
            st = xb_state.setdefault(nb, {"cnt": 0, "ps": None})
            if st["cnt"] >= MT * KE:
                return
            m, k = divmod(st["cnt"], KE)
            if k == 0:
                st["ps"] = (proj_a if xcnt[0] % 2 == 0 else proj_b).tile(
                    [P, 512], F32, tag="gemm", name="ps"
                )
                xcnt[0] += 1
            nc.tensor.matmul(
                st["ps"][:, :512],
                lhsT=wi_sb[:, k, ts(m, P)],
                rhs=in_tail[:, nb, k, :],
                start=(k == 0),
                stop=(k == KE - 1),
            )
            if k == KE - 1:
                nc.vector.tensor_scalar_add(
                    xpT[:, 8 * (nb + 1) : 8 * (nb + 2), ts(m, B)],
                    st["ps"][:, :512].rearrange("p (t b) -> p t b", b=B),
                    bias_sb[:, m : m + 1],
                )
            st["cnt"] += 1

        # head xblock emitted k-outer over 4 live psum tiles so the k0/k1
        # matmuls start on wi's first half while the second half streams in
        head_ps = []
        for m in range(MT):
            ps = (proj_a if m % 2 == 0 else proj_b).tile(
                [P, 512], F32, tag="gemm", name="ps"
            )
            head_ps.append(ps)
        for k in range(KE):
            for m in range(MT):
                nc.tensor.matmul(
                    head_ps[m][:, :128],
                    lhsT=wi_sb[:, k, ts(m, P)],
                    rhs=in_head[:, k, :],
                    start=(k == 0),
                    stop=(k == KE - 1),
                )
        for m in range(MT):
            nc.vector.tensor_scalar_add(
                xpT[:, 0:2, ts(m, B)],
                head_ps[m][:, :128].rearrange("p (t b) -> p t b", b=B),
                bias_sb[:, m : m + 1],
            )
        # a few more warmup matmuls emitted here: the scheduler drops them
        # into the idle gap between the head xblock and the rest xblock
        # (waiting on DMA), keeping the PE clock gate at full rate
        for _ in range(4):
            nc.tensor.matmul(
                wps[:], lhsT=warm[:, 0:P], rhs=warm[:], start=True, stop=True,
                skip_group_check=True,
            )
        xblock(in_resta, None, 2, 128)

        # ---- RNN: hsT[t] = tanh(xpT[t-1] + Wh.T-chunks @ hsT[t-1])
        # h0 = 0, so step 1 is tanh(xpT[0]) with no matmuls.
        nc.scalar.activation(
            hsT[:, :, B : 2 * B],
            xpT[:, 0, :].rearrange("p (m b) -> p m b", b=B),
            mybir.ActivationFunctionType.Tanh,
        )
        # xblock drip windows: block nb's 16 matmuls spread over the steps
        # before its first consumer (step 8nb+9 reads xpT[8nb+8])
        XB_WIN = {t: 0 for t in range(3, 9)}
        XB_WIN.update({t: 1 for t in range(9, 17)})
        XB_WIN.update({t: 2 for t in range(17, 25)})
        for t in range(2, T + 1):
            # restb (xpT[4..8), first needed at step 5) is emitted AFTER
            # step 2 so its 16 matmuls never outrank the wh-gated chain
            # step in the baked priority order
            if t == 3:
                xblock(in_restb, None, 4, 256)
            rp = rnn_psum.tile([P, 512], F32, tag="rnn")
            nc.tensor.matmul(
                rp[:, 0 : MT * B],
                lhsT=ident_sb[:],
                rhs=xpT[:, t - 1, :],
                start=True,
                stop=False,
                skip_group_check=True,
            )
            for m in range(MT):
                for k in range(KH):
                    nc.tensor.matmul(
                        rp[:, ts(m, B)],
                        lhsT=wh_sb[:, k, m, :],
                        rhs=hsT[:, k, (t - 1) * B : t * B],
                        start=False,
                        stop=(k == KH - 1),
                        skip_group_check=True,
                    )
            nc.scalar.activation(
                hsT[:, :, t * B : (t + 1) * B],
                rp[:, 0 : MT * B].rearrange("p (m b) -> p m b", b=B),
                mybir.ActivationFunctionType.Tanh,
            )
            # drip-feed filler matmuls into this step's ~735ns tanh
            # window, budgeted by matmul column-time (512 cols = 215ns)
            # rather than count, so the window is filled exactly: pending
            # xblock work first, leftover budget goes to projection
            # (wy lands ~22us in; earlier proj would head-block the chain)
            fill_ns = 765.0
            if t in XB_WIN:
                nbw = XB_WIN[t]
                while fill_ns > 60 and xb_state.get(nbw, {"cnt": 0})["cnt"] < MT * KE:
                    emit_xb_mm(nbw)
                    fill_ns -= 215.0
            if t >= 10:
                while fill_ns > 60:
                    cost = proj_next_cost()
                    if cost is None or not emit_proj_mm(True):
                        break
                    fill_ns -= cost

        # post-chain drain of the remaining projection
        while emit_proj_mm(False):
            pass

    nc.compile()
    return nc


def make_in_maps(features, captions, embed_table, Wi, bi, Wh, bh, Wy, by):
    f32, f16 = np.float32, np.float16
    emb = np.asarray(embed_table, f32)[np.asarray(captions, np.int64)]  # (B,31,E)
    inputs = np.concatenate(
        [np.asarray(features, f32)[:, None, :], emb], axis=1
    )  # (B,T,E)
    inp_bt = np.ascontiguousarray(inputs.transpose(1, 0, 2).reshape(BT, E))
    # [p, nb, k, c] = inp_bt[nb*512 + c, k*128 + p]
    inpT = np.ascontiguousarray(
        inp_bt.reshape(NB, 512, KE, P).transpose(3, 0, 2, 1)
    ).astype(f16)
    inp_head = np.ascontiguousarray(inpT[:, 0, :, 0:128])
    inp_resta = np.ascontiguousarray(inpT[:, 0, :, 128:256])
    inp_restb = np.ascontiguousarray(inpT[:, 0, :, 256:512])
    inp_tail = np.ascontiguousarray(inpT[:, 1:, :, :])
    wi_h = np.ascontiguousarray(
        np.asarray(Wi, f32).reshape(KE, P, H).transpose(1, 0, 2)
    ).astype(f16)
    wh_h = np.ascontiguousarray(
        np.asarray(Wh, f32).reshape(KH, P, MT, P).transpose(1, 0, 2, 3)
    ).astype(f16)
    bias_h = np.ascontiguousarray(
        (np.asarray(bi, f32) + np.asarray(bh, f32)).reshape(MT, P).T
    )
    wy_f = np.asarray(Wy, f32)
    in_maps = []
    for c in range(NCORES):
        wy_h = np.ascontiguousarray(
            wy_f[:, c * VS : (c + 1) * VS].reshape(KH, P, VS).transpose(1, 0, 2)
        ).astype(f16)
        in_maps.append(
            {
                "inp_head": inp_head,
                "inp_resta": inp_resta,
                "inp_restb": inp_restb,
                "inp_tail": inp_tail,
                "wi": wi_h,
                "wh": wh_h,
                "bias": bias_h,
                "wy": wy_h,
                "ident": np.eye(P, dtype=f16),
            }
        )
    return in_maps


def assemble(core_outs, by):
    full = np.concatenate([np.asarray(o) for o in core_outs], axis=1)  # [BT,V] f16
    res = full.astype(np.float32) + np.asarray(by, np.float32)[None, :]
    return np.ascontiguousarray(
        res.reshape(T, B, V).transpose(1, 0, 2)
    )


def kernel(**inputs) -> np.ndarray:
    in_maps = make_in_maps(**inputs)
    nc = build_program()
    res = run_bass_kernel_spmd(nc, in_maps, core_ids=list(range(NCORES)))
    return assemble([r["out"] for r in res.results], inputs["by"])


# revision 50
# speedup vs baseline: 1.0406x; 1.0406x over previous
"""DecoderRNN Trainium2 kernel (v4).

Math (reference):
    emb = embed_table[captions]                      # (B, 31, E)
    inputs = concat([features[:,None,:], emb], 1)    # (B, T=32, E)
    xproj = inputs @ Wi + (bi + bh)                  # (B, T, H)
    h_t = tanh(xproj_t + h_{t-1} @ Wh)               # scan over T
    out = hs @ Wy + by                               # (B, T, V)

Distribution: vocab-parallel output projection across 8 cores (Wy sharded by
1250 columns); the input GEMM and serial RNN are replicated full-batch on
every core. No collectives. `by` is added on the HOST during assemble, so
every projection psum evacuation is a plain dtype-converting copy that can
run on either DVE or ACT.

v4 = v2's chain structure + three fixes derived from the v2/v3 traces:
  * v2's tail (36.5us at 66% PE busy) was caused by the projection sharing
    one psum pool with the xblock tails: pool slots recycle in allocation
    order, so every projection chunk transitively waited on the t=25 xblock.
    v4 gives the projection its own 4-bank pool (+2 xblock, +2 RNN), and
    emits it after the chain (higher bass_priority number = filler), which
    lets the greedy scheduler pack it into chain idle from step ~3 on.
    (v3 showed the opposite order head-blocks the chain: interleaving the
    projection BEFORE later chain steps gives it lower priority and the PE
    grinds filler while ACT waits.)
  * Evacuation split: DVE carries most psum->sbuf copies, ACT takes chunks
    of the last tiles (tanhs are emitted earlier == lower priority, so ACT
    always prefers the chain). GPSIMD cannot read PSUM at all.
  * Input DMAs split across both HWDGE queues: Sync carries chain-critical
    tensors (wi, inp_head, ident, bias, wh -- wh now BEFORE the bulk, v2
    lost ~2us of chain stall to wh landing after in_rest), ACT carries the
    bulk (in_rest, tails, wy). Output stores (1 per tile) go on Sync.
  * RNN step: identity matmul streams xpT[t-1] into psum (start=True over
    all four [128,64] m-regions), 16 Wh matmuls accumulate, one fused tanh
    (ACT) evacuates psum to hsT. Serial chain is PE->ACT->PE (~1.27us/step),
    which stays under the PE-throughput-bound steady-state cadence.
  * ~10 garbage warmup matmuls bridge the preamble->first-DMA window so the
    PE clock gate is at full rate when the real GEMMs start.
  * fp16 operands everywhere (peak 0.417 ns/col); psum accumulates fp32;
    fp16 output (host upcasts and adds by).

On-chip layout keeps H (or E) on the partition axis everywhere:
    inputsT  [128, nb, k, c]   k = E/128 chunk, c = bt col within nb block
    xpT      [128, t, m*64+b]  m = H/128 chunk of the output
    hsT      [128, m, t*64+b]  t = 1..32 (slot 0 unused)
"""

import sys

sys.path.insert(0, "/opt/trn_rl_repo")

from contextlib import ExitStack

import numpy as np

import concourse.bass as bass
import concourse.mybir as mybir
import concourse.tile as tile
from concourse import bacc
from concourse.bass import ts
from concourse.bass_utils import run_bass_kernel_spmd

B, T, E, H, V = 64, 32, 512, 512, 10000
NCORES = 8
VS = V // NCORES          # vocab shard per core
BT = B * T                # 2048 rows, t-major: row = t*64 + b
P = 128
KE = E // P               # 4 contraction chunks over E
KH = H // P               # 4 contraction chunks over H
MT = H // P               # 4 output chunks of H
NB = 4                    # bt blocks of 512 for the input GEMM
F32 = mybir.dt.float32
F16 = mybir.dt.float16
HOST_BY = True            # by is added on the host in assemble()

# projection N-chunks (psum bank holds 512 fp32 per partition)
VCHUNKS = [(0, 512), (512, 512), (1024, 226)]
assert sum(n for _, n in VCHUNKS) == VS


def build_program() -> bass.Bass:
    nc = bacc.Bacc()

    inp_head = nc.dram_tensor("inp_head", [P, KE, 128], F16, kind="ExternalInput")
    inp_resta = nc.dram_tensor("inp_resta", [P, KE, 128], F16, kind="ExternalInput")
    inp_restb = nc.dram_tensor("inp_restb", [P, KE, 256], F16, kind="ExternalInput")
    inp_tail = nc.dram_tensor("inp_tail", [P, NB - 1, KE, 512], F16, kind="ExternalInput")
    wi = nc.dram_tensor("wi", [P, KE, H], F16, kind="ExternalInput")
    wh = nc.dram_tensor("wh", [P, KH, MT, P], F16, kind="ExternalInput")
    bias = nc.dram_tensor("bias", [P, MT], F32, kind="ExternalInput")  # bi + bh
    wy = nc.dram_tensor("wy", [P, KH, VS], F16, kind="ExternalInput")
    ident = nc.dram_tensor("ident", [P, P], F16, kind="ExternalInput")
    out = nc.dram_tensor("out", [BT, VS], F16, kind="ExternalOutput")

    with ExitStack() as ctx:
        tc = ctx.enter_context(tile.TileContext(nc))
        persist = ctx.enter_context(tc.tile_pool(name="persist", bufs=1))
        out_pool = ctx.enter_context(tc.tile_pool(name="outs", bufs=6))
        proj_a = ctx.enter_context(tc.tile_pool(name="pj_a", bufs=3, space="PSUM"))
        proj_b = ctx.enter_context(tc.tile_pool(name="pj_b", bufs=3, space="PSUM"))
        rnn_psum = ctx.enter_context(tc.tile_pool(name="rn_ps", bufs=2, space="PSUM"))

        # ---- One sync HWDGE queue, ordered by need-time (the two HWDGE
        # queues share DMA bandwidth, and >8 in-flight DMAs stall the issue
        # pipeline on semaphore reuse, so: 8 issues on sync, and the tiny
        # ident/bias via the gpsimd software-DGE queue).
        wi_sb = persist.tile([P, KE, H], F16, tag="wi")
        in_head = persist.tile([P, KE, 128], F16, tag="in_head")
        ident_sb = persist.tile([P, P], F16, tag="ident")
        bias_sb = persist.tile([P, MT], F32, tag="bias")
        wh_sb = persist.tile([P, KH, MT, P], F16, tag="wh")
        in_resta = persist.tile([P, KE, 128], F16, tag="in_resta")
        in_restb = persist.tile([P, KE, 256], F16, tag="in_restb")
        in_tail = persist.tile([P, NB - 1, KE, 512], F16, tag="in_tail")
        wy_sb = persist.tile([P, KH, VS], F16, tag="wy")
        nc.sync.dma_start(out=wi_sb[:, 0:2], in_=wi[:, 0:2])
        nc.sync.dma_start(out=in_head[:], in_=inp_head[:])
        nc.sync.dma_start(out=wi_sb[:, 2:4], in_=wi[:, 2:4])
        nc.sync.dma_start(out=ident_sb[:], in_=ident[:])
        nc.sync.dma_start(out=bias_sb[:], in_=bias[:])
        nc.sync.dma_start(out=in_resta[:], in_=inp_resta[:])
        nc.sync.dma_start(out=wh_sb[:], in_=wh[:])
        nc.sync.dma_start(out=in_restb[:], in_=inp_restb[:])
        for nb in range(NB - 1):
            nc.sync.dma_start(out=in_tail[:, nb, :, :], in_=inp_tail[:, nb, :, :])
        nc.sync.dma_start(out=wy_sb[:], in_=wy[:])

        # ---- HAM warmup: the PE clock-gate throttles to half rate when the
        # array is idle; keep it busy on garbage during the DMA wait.
        warm = persist.tile([P, 512], F16, tag="warm")
        nc.vector.memset(warm[:], 0.0)
        wps = proj_a.tile([P, 512], F32, tag="gemm")
        for _ in range(5):
            nc.tensor.matmul(
                wps[:], lhsT=warm[:, 0:P], rhs=warm[:], start=True, stop=True,
                skip_group_check=True,
            )

        xpT = persist.tile([P, T, MT * B], F16, tag="xpT")
        hsT = persist.tile([P, MT, (T + 1) * B], F16, tag="hsT")

        # ---- projection emitter: out[bt_tile] = hs @ Wy (by added on
        # host), broken into single matmuls so the chain loop can emit a
        # measured 3 per step (fits the real ~740ns tanh window; the Tile
        # scheduler bakes its simulated order into semaphores, so packing
        # must be done at emission). In-chain chunks all use stream A
        # (psum pool proj_a + DVE evac, keeping ACT free for tanhs); the
        # post-chain drain alternates two independent pool+engine streams
        # so one stream's psum-release latency hides under the other.
        pstate = {"pp": None, "osb": None, "cnt": 0}

        def proj_next_cost():
            i, rem = divmod(pstate["cnt"], 12)
            if i >= BT // P:
                return None
            return VCHUNKS[rem // KH][1] * 0.417 + 2.0

        def emit_proj_mm(in_chain):
            cnt = pstate["cnt"]
            i, rem = divmod(cnt, 12)          # 3 chunks x 4 k per tile
            ci, k = divmod(rem, KH)
            if i >= BT // P:
                return False
            v0, vn = VCHUNKS[ci]
            # in-chain chunks stay on stream A (DVE evac; ACT is doing the
            # chain tanhs); post-chain chunks rotate over THREE pools -- the
            # rnn banks are dead after the chain, so they serve a 3rd stream
            st = 0 if in_chain else (i * 3 + ci) % 3
            if k == 0:
                if ci == 0:
                    pstate["osb"] = out_pool.tile(
                        [P, VS], F16, tag="osb", name="osb"
                    )
                if st == 2:
                    pstate["pp"] = rnn_psum.tile(
                        [P, 512], F32, tag="rnn", name="pp"
                    )
                else:
                    pstate["pp"] = (proj_a if st == 0 else proj_b).tile(
                        [P, 512], F32, tag="gemm", name="pp"
                    )
            pp, osb = pstate["pp"], pstate["osb"]
            nc.tensor.matmul(
                pp[:, :vn],
                lhsT=hsT[:, k, (2 * i + 1) * B : (2 * i + 1) * B + P],
                rhs=wy_sb[:, k, v0 : v0 + vn],
                start=(k == 0),
                stop=(k == KH - 1),
            )
            if k == KH - 1:
                if st == 1:
                    nc.scalar.copy(osb[:, v0 : v0 + vn], pp[:, :vn])
                else:
                    nc.vector.tensor_scalar_add(
                        osb[:, v0 : v0 + vn], pp[:, :vn], 0.0
                    )
                if i < BT // P - 1:
                    if ci == 2:
                        nc.sync.dma_start(out=out[ts(i, P), :], in_=osb[:])
                else:
                    # last tile: store each chunk as it lands so the final
                    # transfer is the short 226-col one
                    nc.sync.dma_start(
                        out=out[ts(i, P), v0 : v0 + vn], in_=osb[:, v0 : v0 + vn]
                    )
            pstate["cnt"] = cnt + 1
            return True

        # ---- xprojT = (inputs @ Wi).T + (bi + bh), in bt-blocks; evacuated
        # by DVE (tensor_scalar add of the per-partition bias chunk). The
        # m-chunks alternate between the two psum pools so the real ~1.7us
        # psum-release turnaround never stalls an xblock.
        xcnt = [0]

        def xblock(rhs_tile, nb, t0, cn):
            for m in range(MT):
                ps = (proj_a if xcnt[0] % 2 == 0 else proj_b).tile(
                    [P, 512], F32, tag="gemm", name="ps"
                )
                xcnt[0] += 1
                for k in range(KE):
                    nc.tensor.matmul(
                        ps[:, :cn],
                        lhsT=wi_sb[:, k, ts(m, P)],
                        rhs=rhs_tile[:, k, :] if nb is None else rhs_tile[:, nb, k, :],
                        start=(k == 0),
                        stop=(k == KE - 1),
                    )
                nc.vector.tensor_scalar_add(
                    xpT[:, t0 : t0 + cn // B, ts(m, B)],
                    ps[:, :cn].rearrange("p (t b) -> p t b", b=B),
                    bias_sb[:, m : m + 1],
                )

        # tail-block xblocks as single-matmul emitters so the chain loop can
        # drip 3 per step (a 16-matmul dump at one step stalls the chain
        # ~2.5-4us; the sim's tighter chain model can't hide it)
        xb_state = {}

        def emit_xb_mm(nb):
            st = xb_state.setdefault(nb, {"cnt": 0, "ps": None})
            if st["cnt"] >= MT * KE:
                return
            m, k = divmod(st["cnt"], KE)
            if k == 0:
                st["ps"] = (proj_a if xcnt[0] % 2 == 0 else proj_b).tile(
                    [P, 512], F32, tag="gemm", name="ps"
                )
                xcnt[0] += 1
            nc.tensor.matmul(
                st["ps"][:, :512],
                lhsT=wi_sb[:, k, ts(m, P)],
                rhs=in_tail[:, nb, k, :],
                start=(k == 0),
                stop=(k == KE - 1),
            )
            if k == KE - 1:
                nc.vector.tensor_scalar_add(
                    xpT[:, 8 * (nb + 1) : 8 * (nb + 2), ts(m, B)],
                    st["ps"][:, :512].rearrange("p (t b) -> p t b", b=B),
                    bias_sb[:, m : m + 1],
                )
            st["cnt"] += 1

        # head xblock emitted k-outer over 4 live psum tiles so the k0/k1
        # matmuls start on wi's first half while the second half streams in
        head_ps = []
        for m in range(MT):
            ps = (proj_a if m % 2 == 0 else proj_b).tile(
                [P, 512], F32, tag="gemm", name="ps"
            )
            head_ps.append(ps)
        for k in range(KE):
            for m in range(MT):
                nc.tensor.matmul(
                    head_ps[m][:, :128],
                    lhsT=wi_sb[:, k, ts(m, P)],
                    rhs=in_head[:, k, :],
                    start=(k == 0),
                    stop=(k == KE - 1),
                )
        for m in range(MT):
            nc.vector.tensor_scalar_add(
                xpT[:, 0:2, ts(m, B)],
                head_ps[m][:, :128].rearrange("p (t b) -> p t b", b=B),
                bias_sb[:, m : m + 1],
            )
        # a few more warmup matmuls emitted here: the scheduler drops them
        # into the idle gap between the head xblock and the rest xblock
        # (waiting on DMA), keeping the PE clock gate at full rate
        for _ in range(4):
            nc.tensor.matmul(
                wps[:], lhsT=warm[:, 0:P], rhs=warm[:], start=True, stop=True,
                skip_group_check=True,
            )
        xblock(in_resta, None, 2, 128)

        # ---- RNN: hsT[t] = tanh(xpT[t-1] + Wh.T-chunks @ hsT[t-1])
        # h0 = 0, so step 1 is tanh(xpT[0]) with no matmuls.
        nc.scalar.activation(
            hsT[:, :, B : 2 * B],
            xpT[:, 0, :].rearrange("p (m b) -> p m b", b=B),
            mybir.ActivationFunctionType.Tanh,
        )
        # xblock drip windows: block nb's 16 matmuls spread over the steps
        # before its first consumer (step 8nb+9 reads xpT[8nb+8])
        XB_WIN = {t: 0 for t in range(3, 9)}
        XB_WIN.update({t: 1 for t in range(9, 17)})
        XB_WIN.update({t: 2 for t in range(17, 25)})
        for t in range(2, T + 1):
            # restb (xpT[4..8), first needed at step 5) is emitted AFTER
            # step 2 so its 16 matmuls never outrank the wh-gated chain
            # step in the baked priority order
            if t == 3:
                xblock(in_restb, None, 4, 256)
            rp = rnn_psum.tile([P, 512], F32, tag="rnn")
            nc.tensor.matmul(
                rp[:, 0 : MT * B],
                lhsT=ident_sb[:],
                rhs=xpT[:, t - 1, :],
                start=True,
                stop=False,
                skip_group_check=True,
            )
            for m in range(MT):
                for k in range(KH):
                    nc.tensor.matmul(
                        rp[:, ts(m, B)],
                        lhsT=wh_sb[:, k, m, :],
                        rhs=hsT[:, k, (t - 1) * B : t * B],
                        start=False,
                        stop=(k == KH - 1),
                        skip_group_check=True,
                    )
            nc.scalar.activation(
                hsT[:, :, t * B : (t + 1) * B],
                rp[:, 0 : MT * B].rearrange("p (m b) -> p m b", b=B),
                mybir.ActivationFunctionType.Tanh,
            )
            # drip-feed filler matmuls into this step's ~735ns tanh
            # window, budgeted by matmul column-time (512 cols = 215ns)
            # rather than count, so the window is filled exactly: pending
            # xblock work first, leftover budget goes to projection
            # (wy lands ~22us in; earlier proj would head-block the chain)
            fill_ns = 765.0
            if t in XB_WIN:
                nbw = XB_WIN[t]
                while fill_ns > 60 and xb_state.get(nbw, {"cnt": 0})["cnt"] < MT * KE:
                    emit_xb_mm(nbw)
                    fill_ns -= 215.0
            if t >= 10:
                while fill_ns > 60:
                    cost = proj_next_cost()
                    if cost is None or not emit_proj_mm(True):
                        break
                    fill_ns -= cost

        # post-chain drain of the remaining projection
        while emit_proj_mm(False):
            pass

    nc.compile()
    return nc


def make_in_maps(features, captions, embed_table, Wi, bi, Wh, bh, Wy, by):
    f32, f16 = np.float32, np.float16
    emb = np.asarray(embed_table, f32)[np.asarray(captions, np.int64)]  # (B,31,E)
    inputs = np.concatenate(
        [np.asarray(features, f32)[:, None, :], emb], axis=1
    )  # (B,T,E)
    inp_bt = np.ascontiguousarray(inputs.transpose(1, 0, 2).reshape(BT, E))
    # [p, nb, k, c] = inp_bt[nb*512 + c, k*128 + p]
    inpT = np.ascontiguousarray(
        inp_bt.reshape(NB, 512, KE, P).transpose(3, 0, 2, 1)
    ).astype(f16)
    inp_head = np.ascontiguousarray(inpT[:, 0, :, 0:128])
    inp_resta = np.ascontiguousarray(inpT[:, 0, :, 128:256])
    inp_restb = np.ascontiguousarray(inpT[:, 0, :, 256:512])
    inp_tail = np.ascontiguousarray(inpT[:, 1:, :, :])
    wi_h = np.ascontiguousarray(
        np.asarray(Wi, f32).reshape(KE, P, H).transpose(1, 0, 2)
    ).astype(f16)
    wh_h = np.ascontiguousarray(
        np.asarray(Wh, f32).reshape(KH, P, MT, P).transpose(1, 0, 2, 3)
    ).astype(f16)
    bias_h = np.ascontiguousarray(
        (np.asarray(bi, f32) + np.asarray(bh, f32)).reshape(MT, P).T
    )
    wy_f = np.asarray(Wy, f32)
    in_maps = []
    for c in range(NCORES):
        wy_h = np.ascontiguousarray(
            wy_f[:, c * VS : (c + 1) * VS].reshape(KH, P, VS).transpose(1, 0, 2)
        ).astype(f16)
        in_maps.append(
            {
                "inp_head": inp_head,
                "inp_resta": inp_resta,
                "inp_restb": inp_restb,
                "inp_tail": inp_tail,
                "wi": wi_h,
                "wh": wh_h,
                "bias": bias_h,
                "wy": wy_h,
                "ident": np.eye(P, dtype=f16),
            }
        )
    return in_maps


def assemble(core_outs, by):
    full = np.concatenate([np.asarray(o) for o in core_outs], axis=1)  # [BT,V] f16
    res = full.astype(np.float32) + np.asarray(by, np.float32)[None, :]
    return np.ascontiguousarray(
        res.reshape(T, B, V).transpose(1, 0, 2)
    )


def kernel(**inputs) -> np.ndarray:
    in_maps = make_in_maps(**inputs)
    nc = build_program()
    res = run_bass_kernel_spmd(nc, in_maps, core_ids=list(range(NCORES)))
    return assemble([r["out"] for r in res.results], inputs["by"])
